# revision 1
# baseline (speedup 1.0000x reference)
"""Deformable multi-head sparse attention (DMSA) Bass kernel for Trainium2.

Contract: kernel(**inputs) takes the FULL unsharded inputs (as produced by
setup_inputs()) and returns the FULL output (B, 384, 56, 56) float32.
Internally shards batch B=8 across 8 NeuronCores (pure data parallel,
no collectives), one batch element per core.

Self-contained: hardcodes all shapes; does not read any sibling files.
"""
import sys

for _p in ("/opt/trn_rl_repo", "/opt/pypackages"):
    if _p not in sys.path:
        sys.path.insert(0, _p)

import numpy as np

import concourse.bass as bass
import concourse.mybir as mybir
import concourse.tile as tile
from concourse import bacc
from concourse import bass_utils

F32 = mybir.dt.float32
F32R = mybir.dt.float32r
I16 = mybir.dt.int16
I32 = mybir.dt.int32
AF = mybir.ActivationFunctionType
OP = mybir.AluOpType

# problem constants
B = 8
DIM = 384
DIM_HEAD = 64
NUM_HEAD = 6
G = 3            # deformable groups
NGD = 128        # channels per group
H = 56
W = 56
HW = H * W       # 3136
HO = 28
WO = 28
L = HO * WO      # 784
SCALE = DIM_HEAD ** -0.5
BN_EPS = 1e-6
A = (W - 1) / WO   # 55/28, same for y since H==W and HO==WO
PADD = 60          # padded dwconv input edge (56 + 2*2)

QC = 448           # q-position chunk (free dim of attention matmuls)
NQC = HW // QC     # 7
LC = 112           # kv-position chunk (partition dim of S^T)
NLC = L // LC      # 7


def _r(ap):
    return ap


def build_nc(gelu_exact: bool = True, stop_after: str = ""):
    """Build the per-core Bass program (SPMD: same NEFF on all 8 cores)."""
    nc = bacc.Bacc("TRN2", target_bir_lowering=False, debug=False, num_devices=B)

    din = {}
    def dt_in(name, shape, dtype=F32):
        din[name] = nc.dram_tensor(name, shape, dtype, kind="ExternalInput").ap()
        return din[name]

    dt_in("x", [DIM, HW])
    dt_in("qw_t", [DIM, DIM])
    dt_in("kwk_t", [DIM, DIM])
    dt_in("kwv_t", [DIM, DIM])
    dt_in("pw_t", [NGD, 3])
    dt_in("projw_t", [DIM, DIM])
    dt_in("projb_rs", [NGD, 3])
    dt_in("dww", [NGD, 25])
    dt_in("bn_s", [NGD, 1])
    dt_in("bn_t", [NGD, 1])
    dt_in("ident", [128, 128])
    dt_in("ytab", [LC, 21])
    dt_in("xtab", [LC, 21])

    out_d = nc.dram_tensor("out", [DIM, HW], F32, kind="ExternalOutput").ap()

    with tile.TileContext(nc) as tc:
        _body(nc, tc, din, out_d, gelu_exact, stop_after)

    nc.compile()
    return nc


def _body(nc, tc, din, out_d, gelu_exact, stop_after=""):
    import contextlib
    ctx = contextlib.ExitStack()
    with ctx:
        # persistent pools (whole kernel)
        wpool = ctx.enter_context(tc.tile_pool(name="wpool", bufs=1))
        spool = ctx.enter_context(tc.tile_pool(name="spool", bufs=1))
        qpool = ctx.enter_context(tc.tile_pool(name="qpool", bufs=1))
        psum = ctx.enter_context(tc.tile_pool(name="psum", bufs=1, space="PSUM"))
        dram = ctx.enter_context(tc.tile_pool(name="dram", bufs=1, space="DRAM"))

        # ---------------- phase A: weight loads ----------------
        # fp32r matmul operands must be produced by compute ops (DMA does not
        # round to fp32r), so every DMA-loaded matmul operand goes through a
        # conversion copy into an F32R tile.
        def load_small(key, shape, dtype=F32):
            t = spool.tile(shape, dtype, name=key + "_sb")
            nc.sync.dma_start(t[:], din[key][:])
            return t

        pjb_sb = load_small("projb_rs", [NGD, 3])
        dww_sb = load_small("dww", [NGD, 25])
        bns_sb = load_small("bn_s", [NGD, 1])
        bnt_sb = load_small("bn_t", [NGD, 1])
        idn_sb = load_small("ident", [128, 128])
        ytab_sb = load_small("ytab", [LC, 21])
        xtab_sb = load_small("xtab", [LC, 21])

        # x pool: released after the gather phase
        xctx = contextlib.ExitStack()
        xpool = xctx.enter_context(tc.tile_pool(name="xpool", bufs=1))
        x_sb = [xpool.tile([128, HW], F32R, name=f"x_sb{g}") for g in range(G)]
        qw_sb, kwk_sb, kwv_sb, pjw_sb = [], [], [], []
        with tc.tile_pool(name="ldpool", bufs=1) as ldpool:
            for g in range(G):
                xt = ldpool.tile([128, HW], F32, tag="xtmp", bufs=2, name="xt")
                nc.sync.dma_start(xt[:], din["x"][128 * g:128 * (g + 1), :])
                nc.scalar.activation(x_sb[g][:], xt[:], AF.Copy)
            for name, key, dst in (("qw", "qw_t", qw_sb), ("kwk", "kwk_t", kwk_sb),
                                   ("kwv", "kwv_t", kwv_sb), ("pjw", "projw_t", pjw_sb)):
                for kc in range(3):
                    wt = ldpool.tile([128, DIM], F32, tag="wtmp", bufs=4, name="wt")
                    nc.sync.dma_start(wt[:], din[key][128 * kc:128 * (kc + 1), :])
                    t = wpool.tile([128, DIM], F32R, name=f"{name}_r{kc}")
                    nc.vector.tensor_copy(t[:], wt[:])
                    dst.append(t)
            pw_sb = spool.tile([NGD, 3], F32, name="pw_sb")
            nc.sync.dma_start(pw_sb[:], din["pw_t"][:])

        ones64 = spool.tile([1, 64], F32R, name="ones64")
        nc.vector.memset(ones64[:].bitcast(F32), 1.0)
        ones128 = spool.tile([1, 128], F32R, name="ones128")
        nc.vector.memset(ones128[:].bitcast(F32), 1.0)

        # ---------------- phase B: q = q_w @ x ----------------
        q_sb = [qpool.tile([128, HW], F32R, name=f"q_sb{m}") for m in range(3)]
        for m in range(3):
            for n in range(NQC):
                pq = psum.tile([128, QC], F32, tag="big", bufs=2, name="pq")
                for kc in range(3):
                    nc.tensor.matmul(
                        pq[:],
                        _r(qw_sb[kc][:, 128 * m:128 * (m + 1)]),
                        _r(x_sb[kc][:, QC * n:QC * (n + 1)]),
                        start=(kc == 0), stop=(kc == 2),
                    )
                nc.vector.tensor_copy(q_sb[m][:, QC * n:QC * (n + 1)], pq[:])

        def _dump(tiles):
            for mm, tt in enumerate(tiles[:3]):
                nc.sync.dma_start(out_d[128 * mm:128 * (mm + 1), 0:tt.shape[1]],
                                  tt[:].bitcast(F32) if tt.dtype != F32 else tt[:])

        if stop_after == "B":
            _dump(q_sb)
            xctx.close()
            return

        # ---------------- phases C..G: per-group pipelined ----------------
        idx_dr = dram.tile([G * 4 * NLC * LC], I16)    # flat (g, r, c, p)
        wgt_dr = dram.tile([G * 4 * NLC * LC], F32)
        idx_v = idx_dr.rearrange("(g p r c) -> g p r c", g=G, p=0, r=4, c=NLC) \
            if False else idx_dr.rearrange("(g r c p) -> g p r c", g=G, r=4, c=NLC)
        wgt_v = wgt_dr.rearrange("(g r c p) -> g p r c", g=G, r=4, c=NLC)
        wrap_v = idx_dr.rearrange("(g s q) -> g q s", g=G, q=16)
        wrow_v = wgt_dr.rearrange("(g r n) -> g r n", g=G, r=4)

        xs_sb = [qpool.tile([128, L], F32R, name=f"xs_sb{g}") for g in range(G)]
        idxw = [spool.tile([128, 196], I16, name=f"idxw{g}") for g in range(G)]

        dgctx = __import__("contextlib").ExitStack()
        dgpool = dgctx.enter_context(tc.tile_pool(name="dgpool", bufs=1))
        diag = dgpool.tile([128, 25 * 128], F32R, name="diag")
        for t in range(25):
            nc.scalar.activation(
                diag[:, 128 * t:128 * (t + 1)], idn_sb[:], AF.Copy,
                scale=dww_sb[:, t:t + 1],
            )

        with tc.tile_pool(name="cpool", bufs=1) as cpool:
            def ctile(shape, dtype, tag, bufs=2):
                return cpool.tile(shape, dtype, tag=tag, bufs=bufs, name=tag)

            for g in range(G):
                # --- C1: padded input ---
                pad = ctile([128, PADD * PADD], F32R, "pad", bufs=2)
                nc.vector.memset(pad[:].bitcast(F32), 0.0)
                pad_v = pad[:].rearrange("p (h w) -> p h w", w=PADD)
                qv = q_sb[g][:].rearrange("p (h w) -> p h w", w=W)
                nc.vector.tensor_copy(pad_v[:, 2:58, 2:58], qv[:])

                # --- C3+C4: depthwise conv + BN + GELU ---
                gelu = ctile([128, L], F32, "gelu", bufs=2)
                for nn in range(2):
                    pdw = psum.tile([128, 392], F32, tag="big", bufs=2, name="pdw")
                    for t in range(25):
                        ty, tx = t // 5, t % 5
                        rhs = pad_v[:, ty + 28 * nn: ty + 28 * nn + 28: 2, tx: tx + 56: 2]
                        nc.tensor.matmul(
                            pdw[:], _r(diag[:, 128 * t:128 * (t + 1)]), _r(rhs),
                            start=(t == 0), stop=(t == 24),
                        )
                    gout = gelu[:, 392 * nn:392 * (nn + 1)]
                    if gelu_exact:
                        nc.scalar.activation(gout, pdw[:], AF.Gelu,
                                             bias=bnt_sb[:, 0:1], scale=bns_sb[:, 0:1])
                    else:
                        aa = ctile([128, 392], F32, "simg1")
                        nc.scalar.activation(aa[:], pdw[:], AF.Identity,
                                             bias=bnt_sb[:, 0:1], scale=bns_sb[:, 0:1])
                        ss = ctile([128, 392], F32, "simg2")
                        nc.scalar.activation(ss[:], aa[:], AF.Sigmoid, scale=1.702)
                        nc.vector.tensor_tensor(gout, aa[:], ss[:], op=OP.mult)

                # --- C5: om^T = gelu^T @ pw -> [112 pos, (chunk, ch)] ---
                pom = psum.tile([LC, 21], F32, tag="s", bufs=3, name="pom")
                for c in range(NLC):
                    nc.tensor.matmul(
                        pom[:, 3 * c:3 * (c + 1)],
                        gelu[:, LC * c:LC * (c + 1)],
                        pw_sb[:, 0:3],
                        start=True, stop=True,
                    )
                om_g = ctile([LC, 21], F32, "om_g")
                nc.vector.tensor_copy(om_g[:], pom[:])

                # --- D: position math on [112, 7] slices ---
                om_v = om_g[:].rearrange("p (k ch) -> p k ch", ch=3)
                om0, om1, om2 = om_v[:, :, 0], om_v[:, :, 1], om_v[:, :, 2]
                yt = ytab_sb[:, 0:NLC]
                xt = xtab_sb[:, 0:NLC]

                def dvt(tag):
                    return ctile([LC, NLC], F32, tag)

                ty_t = dvt("ty_t"); tx_t = dvt("tx_t"); mod_t = dvt("mod_t")
                nc.scalar.activation(ty_t[:], om0, AF.Tanh)
                nc.scalar.activation(tx_t[:], om1, AF.Tanh)
                sg_t = dvt("sg_t")
                nc.scalar.activation(sg_t[:], om2, AF.Sigmoid)
                nc.scalar.activation(mod_t[:], sg_t[:], AF.Sigmoid)

                gy2 = dvt("gy2"); gx2 = dvt("gx2")
                nc.vector.tensor_tensor(gy2[:], ty_t[:], yt, op=OP.add)
                nc.vector.tensor_scalar(gy2[:], gy2[:], float(A), None, OP.mult)
                nc.vector.tensor_tensor(gx2[:], tx_t[:], xt, op=OP.add)
                nc.vector.tensor_scalar(gx2[:], gx2[:], float(A), None, OP.mult)

                def floor_of(gt, tag):
                    ii = ctile([LC, NLC], I32, tag + "_i")
                    nc.vector.tensor_copy(ii[:], gt[:])
                    ff = dvt(tag + "_f")
                    nc.vector.tensor_copy(ff[:], ii[:])
                    fxm = dvt(tag + "_fix")
                    nc.vector.tensor_tensor(fxm[:], ff[:], gt[:], op=OP.is_gt)
                    nc.vector.tensor_tensor(ff[:], ff[:], fxm[:], op=OP.subtract)
                    return ff

                y0s = floor_of(gy2, "y0s")
                x0s = floor_of(gx2, "x0s")

                fy = dvt("fy"); fx_ = dvt("fx_")
                nc.vector.tensor_tensor(fy[:], gy2[:], y0s[:], op=OP.subtract)
                nc.vector.tensor_tensor(fx_[:], gx2[:], x0s[:], op=OP.subtract)

                my0 = dvt("my0"); my1 = dvt("my1"); mx0 = dvt("mx0"); mx1 = dvt("mx1")
                nc.vector.tensor_scalar(my0[:], gy2[:], 2.0, None, OP.is_ge)
                nc.vector.tensor_scalar(my1[:], gy2[:], 57.0, None, OP.is_lt)
                nc.vector.tensor_scalar(mx0[:], gx2[:], 2.0, None, OP.is_ge)
                nc.vector.tensor_scalar(mx1[:], gx2[:], 57.0, None, OP.is_lt)

                wy0 = dvt("wy0"); wy1 = dvt("wy1"); wx0 = dvt("wx0"); wx1 = dvt("wx1")
                omf = dvt("omf")
                nc.vector.tensor_scalar(omf[:], fy[:], -1.0, 1.0, OP.mult, OP.add)
                nc.vector.tensor_tensor(wy0[:], omf[:], my0[:], op=OP.mult)
                nc.vector.tensor_tensor(wy0[:], wy0[:], mod_t[:], op=OP.mult)
                nc.vector.tensor_tensor(wy1[:], fy[:], my1[:], op=OP.mult)
                nc.vector.tensor_tensor(wy1[:], wy1[:], mod_t[:], op=OP.mult)
                nc.vector.tensor_scalar(omf[:], fx_[:], -1.0, 1.0, OP.mult, OP.add)
                nc.vector.tensor_tensor(wx0[:], omf[:], mx0[:], op=OP.mult)
                nc.vector.tensor_tensor(wx1[:], fx_[:], mx1[:], op=OP.mult)

                Wt_g = ctile([LC, 4 * NLC], F32, "Wt_g")
                Wv = Wt_g[:].rearrange("p (r c) -> p r c", r=4)
                nc.vector.tensor_tensor(Wv[:, 0, :], wy0[:], wx0[:], op=OP.mult)
                nc.vector.tensor_tensor(Wv[:, 1, :], wy0[:], wx1[:], op=OP.mult)
                nc.vector.tensor_tensor(Wv[:, 2, :], wy1[:], wx0[:], op=OP.mult)
                nc.vector.tensor_tensor(Wv[:, 3, :], wy1[:], wx1[:], op=OP.mult)

                yc0 = dvt("yc0"); yc1 = dvt("yc1"); xc0 = dvt("xc0"); xc1 = dvt("xc1")
                nc.vector.tensor_scalar(yc0[:], y0s[:], -2.0, 0.0, OP.add, OP.max)
                nc.vector.tensor_scalar(yc0[:], yc0[:], 55.0, 56.0, OP.min, OP.mult)
                nc.vector.tensor_scalar(yc1[:], y0s[:], -1.0, 0.0, OP.add, OP.max)
                nc.vector.tensor_scalar(yc1[:], yc1[:], 55.0, 56.0, OP.min, OP.mult)
                nc.vector.tensor_scalar(xc0[:], x0s[:], -2.0, 0.0, OP.add, OP.max)
                nc.vector.tensor_scalar(xc0[:], xc0[:], 55.0, None, OP.min)
                nc.vector.tensor_scalar(xc1[:], x0s[:], -1.0, 0.0, OP.add, OP.max)
                nc.vector.tensor_scalar(xc1[:], xc1[:], 55.0, None, OP.min)

                If_g = ctile([LC, 4 * NLC], F32, "If_g")
                Ifv = If_g[:].rearrange("p (r c) -> p r c", r=4)
                nc.vector.tensor_tensor(Ifv[:, 0, :], yc0[:], xc0[:], op=OP.add)
                nc.vector.tensor_tensor(Ifv[:, 1, :], yc0[:], xc1[:], op=OP.add)
                nc.vector.tensor_tensor(Ifv[:, 2, :], yc1[:], xc0[:], op=OP.add)
                nc.vector.tensor_tensor(Ifv[:, 3, :], yc1[:], xc1[:], op=OP.add)
                Ii_g = ctile([LC, 4 * NLC], I16, "Ii_g")
                nc.vector.tensor_copy(Ii_g[:], If_g[:])

                # --- E: DRAM wrap roundtrip ---
                nc.sync.dma_start(idx_v[g], Ii_g[:])
                nc.sync.dma_start(wgt_v[g], Wt_g[:])
                for gi in range(8):
                    nc.sync.dma_start(idxw[g][16 * gi:16 * (gi + 1), :], wrap_v[g])

                wbc = []
                for r in range(4):
                    wrow_f = ctile([1, L], F32, "wrow_f", bufs=1)
                    nc.sync.dma_start(wrow_f[:], wrow_v[g, r][None, :])
                    wrow = ctile([1, L], F32R, "wrow", bufs=1)
                    nc.vector.tensor_copy(wrow[:], wrow_f[:])
                    t = ctile([128, L], F32, "wbc", bufs=4)
                    for n2 in range(2):
                        pwb = psum.tile([128, 392], F32, tag="big", bufs=2, name="pwb")
                        nc.tensor.matmul(
                            pwb[:], ones128[:],
                            wrow[:, 392 * n2:392 * (n2 + 1)],
                            start=True, stop=True,
                        )
                        nc.scalar.activation(t[:, 392 * n2:392 * (n2 + 1)], pwb[:], AF.Copy)
                    wbc.append(t)

                # --- F+G: gather + bilinear ---
                gat = ctile([128, 4 * L], F32, "gat", bufs=2)
                nc.gpsimd.ap_gather(
                    gat[:], x_sb[g][:].bitcast(F32), idxw[g][:],
                    channels=128, num_elems=HW, d=1, num_idxs=4 * L,
                )
                tmp = ctile([128, L], F32, "biltmp", bufs=1)
                nc.vector.tensor_tensor(xs_sb[g][:], gat[:, 0:L], wbc[0][:], op=OP.mult)
                for r in range(1, 4):
                    nc.vector.tensor_tensor(tmp[:], gat[:, L * r:L * (r + 1)],
                                            wbc[r][:], op=OP.mult)
                    nc.vector.tensor_tensor(xs_sb[g][:], xs_sb[g][:], tmp[:], op=OP.add)

        dgctx.close()
        xctx.close()   # release x tiles
        if stop_after == "G":
            _dump(xs_sb)
            return

        # ---------------- phase H: k and v^T ----------------
        hpool = ctx.enter_context(tc.tile_pool(name="hpool", bufs=1))
        k_sb = [hpool.tile([128, L], F32R, name=f"k_sb{m}") for m in range(3)]
        for m in range(3):
            for n2 in range(2):
                pk = psum.tile([128, 392], F32, tag="big", bufs=2, name="pk")
                for kc in range(3):
                    nc.tensor.matmul(
                        pk[:],
                        _r(kwk_sb[kc][:, 128 * m:128 * (m + 1)]),
                        _r(xs_sb[kc][:, 392 * n2:392 * (n2 + 1)]),
                        start=(kc == 0), stop=(kc == 2),
                    )
                nc.scalar.activation(k_sb[m][:, 392 * n2:392 * (n2 + 1)], pk[:], AF.Copy)

        vTe = [hpool.tile([LC, 6 * 65], F32R, name=f"vTe{lc}") for lc in range(NLC)]
        for lc in range(NLC):
            nc.vector.memset(vTe[lc][:].bitcast(F32), 1.0)
            pv = psum.tile([LC, DIM], F32, tag="big", bufs=2, name="pv")
            for kc in range(3):
                nc.tensor.matmul(
                    pv[:],
                    _r(xs_sb[kc][:, LC * lc:LC * (lc + 1)]),
                    _r(kwv_sb[kc][:, 0:DIM]),
                    start=(kc == 0), stop=(kc == 2),
                )
            dst = vTe[lc][:].rearrange("p (h d) -> p h d", h=6)[:, :, 0:64]
            nc.scalar.activation(dst, pv[:].rearrange("p (h d) -> p h d", h=6), AF.Copy)

        if stop_after == "H":
            _dump(k_sb)
            return

        # ---------------- phases I+J ----------------
        with tc.tile_pool(name="opool", bufs=1) as opool, \
             tc.tile_pool(name="apool", bufs=1) as apool:
            O_all = [opool.tile([128, HW], F32R, name=f"O_all{m}") for m in range(3)]

            def st_phase(h, qi):
                m2, hh = h // 2, h % 2
                Es = []
                for lc in range(NLC):
                    ps_s = psum.tile([LC, QC], F32, tag="s", bufs=3, name="ps_s")
                    nc.tensor.matmul(
                        ps_s[:],
                        _r(k_sb[m2][64 * hh:64 * hh + 64, LC * lc:LC * (lc + 1)]),
                        _r(q_sb[m2][64 * hh:64 * hh + 64, QC * qi:QC * (qi + 1)]),
                        start=True, stop=True,
                    )
                    E = apool.tile([LC, QC], F32R, tag="E", bufs=16, name="E")
                    nc.scalar.activation(E[:], ps_s[:], AF.Exp)
                    Es.append(E)
                return Es

            def ot_phase(h, qi, Es):
                m2, hh = h // 2, h % 2
                ps_o = psum.tile([65, QC], F32, tag="o", bufs=3, name="ps_o")
                for lc in range(NLC):
                    nc.tensor.matmul(
                        ps_o[:],
                        _r(vTe[lc][:, 65 * h:65 * (h + 1)]),
                        _r(Es[lc][:]),
                        start=(lc == 0), stop=(lc == NLC - 1),
                    )
                rec = apool.tile([1, QC], F32R, tag="rec", bufs=4, name="rec")
                with nc.allow_low_precision(reason="f32r is fp32-width"):
                    nc.vector.reciprocal(rec[:], ps_o[64:65, :])
                ps_rb = psum.tile([64, QC], F32, tag="o", bufs=3, name="ps_rb")
                nc.tensor.matmul(ps_rb[:], ones64[:], rec[:],
                                 start=True, stop=True)
                oslice = O_all[m2][64 * hh:64 * hh + 64, QC * qi:QC * (qi + 1)]
                nc.vector.tensor_copy(oslice, ps_o[0:64, :])
                nc.vector.tensor_tensor(oslice, oslice, ps_rb[:], op=OP.mult)

            attn_iters = [(h, qi) for h in range(NUM_HEAD) for qi in range(NQC)]
            if stop_after.startswith("I1"):
                attn_iters = attn_iters[:1]
            pending = None
            for it in attn_iters:
                Es = st_phase(*it)
                if pending is not None:
                    ot_phase(pending[0][0], pending[0][1], pending[1])
                pending = (it, Es)
            if pending is not None:
                ot_phase(pending[0][0], pending[0][1], pending[1])

            if stop_after.startswith("I1") or stop_after == "I":
                _dump(O_all[:1] if stop_after.startswith("I1") else O_all)
                return

            # proj
            for m in range(3):
                for n in range(NQC):
                    pp = psum.tile([128, QC], F32, tag="big", bufs=2, name="pp")
                    for kc in range(3):
                        nc.tensor.matmul(
                            pp[:],
                            _r(pjw_sb[kc][:, 128 * m:128 * (m + 1)]),
                            _r(O_all[kc][:, QC * n:QC * (n + 1)]),
                            start=(kc == 0), stop=(kc == 2),
                        )
                    y = apool.tile([128, QC], F32, tag="y", bufs=3, name="y")
                    nc.vector.tensor_scalar(y[:], pp[:], pjb_sb[:, m:m + 1], None,
                                            OP.add)
                    nc.sync.dma_start(
                        out_d[128 * m:128 * (m + 1), QC * n:QC * (n + 1)], y[:])


def host_prep(inputs):
    """Shared (per-core-identical) weight prep. Returns dict of np arrays."""
    f = np.float32
    q_w = np.asarray(inputs["q_w"], f)
    kv_w = np.asarray(inputs["kv_w"], f)
    proj_w = np.asarray(inputs["proj_w"], f)
    proj_b = np.asarray(inputs["proj_b"], f)
    dw_w = np.asarray(inputs["dw_w"], f)
    dw_b = np.asarray(inputs["dw_b"], f)
    bn_w = np.asarray(inputs["bn_w"], f)
    bn_b = np.asarray(inputs["bn_b"], f)
    bn_mean = np.asarray(inputs["bn_mean"], f)
    bn_var = np.asarray(inputs["bn_var"], f)
    pw_w = np.asarray(inputs["pw_w"], f)

    bn_s = (bn_w / np.sqrt(bn_var + BN_EPS)).astype(f)
    bn_t = ((dw_b - bn_mean) * bn_s + bn_b).astype(f)

    p = np.arange(LC)
    c = np.arange(NLC)
    ytab_col = (4 * c[None, :] + p[:, None] // 28 + 0.5 + 2.0 / A).astype(f)  # [112, 7]
    ytab = np.tile(ytab_col, (1, G))                                          # [112, 21]
    xtab_col = (p % 28 + 0.5 + 2.0 / A).astype(f)[:, None]
    xtab = np.tile(xtab_col, (1, G * NLC))

    return {
        "qw_t": np.ascontiguousarray(q_w.T),
        "kwk_t": np.ascontiguousarray((kv_w[:DIM] * SCALE).T),
        "kwv_t": np.ascontiguousarray(kv_w[DIM:].T),
        "pw_t": np.ascontiguousarray(pw_w.T),
        "projw_t": np.ascontiguousarray(proj_w.T),
        "projb_rs": np.ascontiguousarray(proj_b.reshape(3, NGD).T),
        "dww": np.ascontiguousarray(dw_w.reshape(NGD, 25)),
        "bn_s": bn_s.reshape(NGD, 1),
        "bn_t": bn_t.reshape(NGD, 1),
        "ident": np.eye(128, dtype=f),
        "ytab": ytab,
        "xtab": xtab,
    }


_NC_CACHE = {}


def _get_nc(gelu_exact=True):
    key = bool(gelu_exact)
    if key not in _NC_CACHE:
        _NC_CACHE[key] = build_nc(gelu_exact=key)
    return _NC_CACHE[key]


def make_in_maps(inputs):
    shared = host_prep(inputs)
    x = np.asarray(inputs["x"], np.float32)
    in_maps = []
    for i in range(B):
        m = dict(shared)
        m["x"] = np.ascontiguousarray(x[i].reshape(DIM, HW))
        in_maps.append(m)
    return in_maps


def run_spmd(inputs, trace=False):
    """Run on the 8 NeuronCores; returns (out (8,384,56,56), BassKernelResults)."""
    nc = _get_nc(True)
    in_maps = make_in_maps(inputs)
    res = bass_utils.run_bass_kernel_spmd(
        nc, in_maps, core_ids=list(range(B)), trace=trace,
    )
    out = np.stack([r["out"].reshape(DIM, H, W) for r in res.results], axis=0)
    return out, res


def kernel(**inputs) -> np.ndarray:
    out, _ = run_spmd(inputs, trace=False)
    return out



# revision 31
# speedup vs baseline: 1.4122x; 1.4122x over previous
"""Deformable multi-head sparse attention (DMSA) Bass kernel for Trainium2.

Contract: kernel(**inputs) takes the FULL unsharded inputs (as produced by
setup_inputs()) and returns the FULL output (B, 384, 56, 56) float32.
Internally shards batch B=8 across 8 NeuronCores (pure data parallel,
no collectives), one batch element per core.

Self-contained: hardcodes all shapes; does not read any sibling files.
"""
import sys

for _p in ("/opt/trn_rl_repo", "/opt/pypackages"):
    if _p not in sys.path:
        sys.path.insert(0, _p)

import numpy as np

import concourse.bass as bass
import concourse.mybir as mybir
import concourse.tile as tile
from concourse import bacc
from concourse import bass_utils

F32 = mybir.dt.float32
F32R = mybir.dt.float32r
I16 = mybir.dt.int16
I32 = mybir.dt.int32
AF = mybir.ActivationFunctionType
OP = mybir.AluOpType

# problem constants
B = 8
DIM = 384
DIM_HEAD = 64
NUM_HEAD = 6
G = 3            # deformable groups
NGD = 128        # channels per group
H = 56
W = 56
HW = H * W       # 3136
HO = 28
WO = 28
L = HO * WO      # 784
SCALE = DIM_HEAD ** -0.5
BN_EPS = 1e-6
A = (W - 1) / WO   # 55/28, same for y since H==W and HO==WO
PADD = 60          # padded dwconv input edge (56 + 2*2)

QC = 448           # q-position chunk (free dim of attention matmuls)
NQC = HW // QC     # 7
LC = 112           # kv-position chunk (partition dim of S^T)
NLC = L // LC      # 7


def _r(ap):
    return ap


def build_nc(gelu_exact: bool = True, stop_after: str = ""):
    """Build the per-core Bass program (SPMD: same NEFF on all 8 cores)."""
    nc = bacc.Bacc("TRN2", target_bir_lowering=False, debug=False, num_devices=B)

    din = {}
    def dt_in(name, shape, dtype=F32):
        din[name] = nc.dram_tensor(name, shape, dtype, kind="ExternalInput").ap()
        return din[name]

    dt_in("x", [DIM, HW])
    dt_in("qw_t", [DIM, DIM])
    dt_in("kwk_t", [DIM, DIM])
    dt_in("kwv_t", [DIM, DIM])
    dt_in("pw_t", [NGD, 3])
    dt_in("projw_t", [DIM, DIM])
    dt_in("projb_rs", [NGD, 3])
    dt_in("dww", [NGD, 25])
    dt_in("bn_s", [NGD, 1])
    dt_in("bn_t", [NGD, 1])
    dt_in("ident", [128, 128])
    dt_in("perm", [LC, 7 * 128])
    dt_in("ytab", [LC, 21])
    dt_in("xtab", [LC, 21])

    out_d = nc.dram_tensor("out", [DIM, HW], F32, kind="ExternalOutput").ap()

    with tile.TileContext(nc) as tc:
        _body(nc, tc, din, out_d, gelu_exact, stop_after)

    nc.compile()
    return nc


def _body(nc, tc, din, out_d, gelu_exact, stop_after=""):
    import contextlib
    ctx = contextlib.ExitStack()
    with ctx:
        # persistent pools (whole kernel)
        wpool = ctx.enter_context(tc.tile_pool(name="wpool", bufs=1))
        spool = ctx.enter_context(tc.tile_pool(name="spool", bufs=1))
        qpool = ctx.enter_context(tc.tile_pool(name="qpool", bufs=1))
        psum = ctx.enter_context(tc.tile_pool(name="psum", bufs=1, space="PSUM"))
        dram = ctx.enter_context(tc.tile_pool(name="dram", bufs=1, space="DRAM"))

        # ---------------- phase A: weight loads ----------------
        # fp32r matmul operands must be produced by compute ops (DMA does not
        # round to fp32r), so every DMA-loaded matmul operand goes through a
        # conversion copy into an F32R tile.
        def load_small(key, shape, dtype=F32):
            t = spool.tile(shape, dtype, name=key + "_sb")
            nc.scalar.dma_start(t[:], din[key][:])
            return t

        pjb_sb = load_small("projb_rs", [NGD, 3])
        dww_sb = load_small("dww", [NGD, 25])
        bns_sb = load_small("bn_s", [NGD, 1])
        bnt_sb = load_small("bn_t", [NGD, 1])
        idn_sb = load_small("ident", [128, 128])
        ytab_sb = load_small("ytab", [LC, 21])
        xtab_sb = load_small("xtab", [LC, 21])

        # x pool: released after the gather phase
        xctx = contextlib.ExitStack()
        xpool = xctx.enter_context(tc.tile_pool(name="xpool", bufs=1))
        x_sb = [xpool.tile([128, HW], F32R, name=f"x_sb{g}") for g in range(G)]
        qw_sb, kwk_sb, kwv_sb, pjw_sb = [], [], [], []
        with tc.tile_pool(name="ldpool", bufs=1) as ldpool:
            # x first on the SP DMA queue (it gates the q matmuls); the
            # non-q weights go via the Activation DMA queue in parallel
            for g in range(G):
                xt = ldpool.tile([128, HW], F32, tag="xtmp", bufs=3, name="xt")
                nc.sync.dma_start(xt[:], din["x"][128 * g:128 * (g + 1), :])
                nc.vector.tensor_copy(x_sb[g][:], xt[:])
            for name, key, dst in (("qw", "qw_t", qw_sb), ("kwk", "kwk_t", kwk_sb),
                                   ("kwv", "kwv_t", kwv_sb), ("pjw", "projw_t", pjw_sb)):
                for kc in range(3):
                    wt = ldpool.tile([128, DIM], F32, tag="wtmp", bufs=4, name="wt")
                    nc.scalar.dma_start(wt[:], din[key][128 * kc:128 * (kc + 1), :])
                    t = wpool.tile([128, DIM], F32R, name=f"{name}_r{kc}")
                    nc.vector.tensor_copy(t[:], wt[:])
                    dst.append(t)
            pw_sb = spool.tile([NGD, 3], F32, name="pw_sb")
            nc.scalar.dma_start(pw_sb[:], din["pw_t"][:])
            # idx-wrap permutation matrices (7x [112, 128]), used as f32r lhsT
            pm = ldpool.tile([LC, 7 * 128], F32, tag="pm", bufs=1, name="pm")
            nc.scalar.dma_start(pm[:], din["perm"][:])
            permr = wpool.tile([LC, 7 * 128], F32R, name="permr")
            nc.vector.tensor_copy(permr[:], pm[:])

        # ---------------- phase B: q = q_w @ x ----------------
        q_sb = [qpool.tile([128, HW], F32R, name=f"q_sb{m}") for m in range(3)]
        for m in range(3):
            for n in range(NQC):
                pq = psum.tile([128, QC], F32, tag="big", bufs=2, name="pq")
                for kc in range(3):
                    nc.tensor.matmul(
                        pq[:],
                        _r(qw_sb[kc][:, 128 * m:128 * (m + 1)]),
                        _r(x_sb[kc][:, QC * n:QC * (n + 1)]),
                        start=(kc == 0), stop=(kc == 2),
                    )
                # spread psum evacuation across engines (gpsimd has no PSUM port)
                dstap = q_sb[m][:, QC * n:QC * (n + 1)]
                if n % 2 == 0:
                    nc.vector.tensor_copy(dstap, pq[:])
                else:
                    nc.scalar.activation(dstap, pq[:], AF.Copy)

        def _dump(tiles):
            for mm, tt in enumerate(tiles[:3]):
                nc.sync.dma_start(out_d[128 * mm:128 * (mm + 1), 0:tt.shape[1]],
                                  tt[:].bitcast(F32) if tt.dtype != F32 else tt[:])

        if stop_after == "B":
            _dump(q_sb)
            xctx.close()
            return

        # ---------------- phases C..G: per-group pipelined ----------------
        # wgt DRAM layout: flat (g, r, c, p); written from the PE-transposed
        # [28, 112] tile so both the write (28 runs of 448B) and the per-row
        # reads (contiguous 784) stay descriptor-cheap.
        wgt_dr = dram.tile([G * 4 * NLC * LC], F32)
        wgtT_v = wgt_dr.rearrange("(g rc p) -> g rc p", g=G, rc=4 * NLC)
        wrow_v = wgt_dr.rearrange("(g r n) -> g r n", g=G, r=4)

        xs_sb = [qpool.tile([128, L], F32R, name=f"xs_sb{g}") for g in range(G)]
        idxw = [spool.tile([128, 196], I16, name=f"idxw{g}") for g in range(G)]

        dgctx = contextlib.ExitStack()
        dgpool = dgctx.enter_context(tc.tile_pool(name="dgpool", bufs=1))
        diag = dgpool.tile([128, 25 * 128], F32R, name="diag")
        for t in range(25):
            # diag[:, t] = ident * dww[:, t]  (per-partition scalar broadcast)
            nc.gpsimd.tensor_scalar(
                diag[:, 128 * t:128 * (t + 1)], idn_sb[:],
                dww_sb[:, t:t + 1], None, OP.mult,
            )

        with tc.tile_pool(name="cpool", bufs=1) as cpool:
            def ctile(shape, dtype, tag, bufs=2):
                return cpool.tile(shape, dtype, tag=tag, bufs=bufs, name=tag)

            for g in range(G):
                # --- C1: padded input (zero borders only; interior copied).
                # On Pool/Act: the DVE queue is the group-pipeline bottleneck.
                pad = ctile([128, PADD * PADD], F32R, "pad", bufs=2)
                pad_v = pad[:].rearrange("p (h w) -> p h w", w=PADD)
                pf = pad[:].bitcast(F32)
                pf_v = pf.rearrange("p (h w) -> p h w", w=PADD)
                nc.gpsimd.memset(pf_v[:, 0:2, :], 0.0)
                nc.gpsimd.memset(pf_v[:, 58:60, :], 0.0)
                nc.gpsimd.memset(pf_v[:, 2:58, 0:2], 0.0)
                nc.gpsimd.memset(pf_v[:, 2:58, 58:60], 0.0)
                qv = q_sb[g][:].rearrange("p (h w) -> p h w", w=W)
                nc.scalar.activation(pad_v[:, 2:58, 2:58], qv[:], AF.Copy)

                # --- C3+C4: depthwise conv + BN + GELU ---
                gelu = ctile([128, L], F32, "gelu", bufs=2)
                for nn in range(2):
                    pdw = psum.tile([128, 392], F32, tag="big", bufs=2, name="pdw")
                    for t in range(25):
                        ty, tx = t // 5, t % 5
                        rhs = pad_v[:, ty + 28 * nn: ty + 28 * nn + 28: 2, tx: tx + 56: 2]
                        nc.tensor.matmul(
                            pdw[:], _r(diag[:, 128 * t:128 * (t + 1)]), _r(rhs),
                            start=(t == 0), stop=(t == 24),
                        )
                    gout = gelu[:, 392 * nn:392 * (nn + 1)]
                    nc.scalar.activation(gout, pdw[:], AF.Gelu,
                                         bias=bnt_sb[:, 0:1], scale=bns_sb[:, 0:1])

                # --- C5: om^T = gelu^T @ pw -> [112 pos, (chunk, ch)] ---
                pom = psum.tile([LC, 21], F32, tag="o", bufs=2, name="pom")
                for c in range(NLC):
                    nc.tensor.matmul(
                        pom[:, 3 * c:3 * (c + 1)],
                        gelu[:, LC * c:LC * (c + 1)],
                        pw_sb[:, 0:3],
                        start=True, stop=True,
                    )
                om_g = ctile([LC, 21], F32, "om_g")
                nc.scalar.activation(om_g[:], pom[:], AF.Copy)

                # --- D: position math on [112, 7] slices ---
                om_v = om_g[:].rearrange("p (k ch) -> p k ch", ch=3)
                om0, om1, om2 = om_v[:, :, 0], om_v[:, :, 1], om_v[:, :, 2]
                yt = ytab_sb[:, 0:NLC]
                xt = xtab_sb[:, 0:NLC]

                def dvt(tag):
                    return ctile([LC, NLC], F32, tag)

                # sigmoid(x) = 0.5*tanh(0.5x) + 0.5 (tanh shares the gelu/exp
                # act tables; sigmoid would force table reloads)
                ty_t = dvt("ty_t"); tx_t = dvt("tx_t"); mod_t = dvt("mod_t")
                nc.scalar.activation(ty_t[:], om0, AF.Tanh)
                nc.scalar.activation(tx_t[:], om1, AF.Tanh)
                sg_t = dvt("sg_t")
                nc.scalar.activation(sg_t[:], om2, AF.Tanh, scale=0.5)
                nc.vector.tensor_scalar(sg_t[:], sg_t[:], 0.5, 0.5, OP.mult, OP.add)
                nc.scalar.activation(mod_t[:], sg_t[:], AF.Tanh, scale=0.5)
                nc.vector.tensor_scalar(mod_t[:], mod_t[:], 0.5, 0.5, OP.mult, OP.add)

                gy2 = dvt("gy2"); gx2 = dvt("gx2")
                nc.vector.tensor_tensor(gy2[:], ty_t[:], yt, op=OP.add)
                nc.vector.tensor_scalar(gy2[:], gy2[:], float(A), None, OP.mult)
                nc.vector.tensor_tensor(gx2[:], tx_t[:], xt, op=OP.add)
                nc.vector.tensor_scalar(gx2[:], gx2[:], float(A), None, OP.mult)

                def floor_of(gt, tag):
                    ii = ctile([LC, NLC], I32, tag + "_i")
                    nc.vector.tensor_copy(ii[:], gt[:])
                    ff = dvt(tag + "_f")
                    nc.vector.tensor_copy(ff[:], ii[:])
                    fxm = dvt(tag + "_fix")
                    nc.vector.tensor_tensor(fxm[:], ff[:], gt[:], op=OP.is_gt)
                    nc.vector.tensor_tensor(ff[:], ff[:], fxm[:], op=OP.subtract)
                    return ff

                y0s = floor_of(gy2, "y0s")
                x0s = floor_of(gx2, "x0s")

                fy = dvt("fy"); fx_ = dvt("fx_")
                nc.vector.tensor_tensor(fy[:], gy2[:], y0s[:], op=OP.subtract)
                nc.vector.tensor_tensor(fx_[:], gx2[:], x0s[:], op=OP.subtract)

                my0 = dvt("my0"); my1 = dvt("my1"); mx0 = dvt("mx0"); mx1 = dvt("mx1")
                nc.vector.tensor_scalar(my0[:], gy2[:], 2.0, None, OP.is_ge)
                nc.vector.tensor_scalar(my1[:], gy2[:], 57.0, None, OP.is_lt)
                nc.vector.tensor_scalar(mx0[:], gx2[:], 2.0, None, OP.is_ge)
                nc.vector.tensor_scalar(mx1[:], gx2[:], 57.0, None, OP.is_lt)

                wy0 = dvt("wy0"); wy1 = dvt("wy1"); wx0 = dvt("wx0"); wx1 = dvt("wx1")
                omf = dvt("omf")
                nc.vector.tensor_scalar(omf[:], fy[:], -1.0, 1.0, OP.mult, OP.add)
                nc.vector.tensor_tensor(wy0[:], omf[:], my0[:], op=OP.mult)
                nc.vector.tensor_tensor(wy0[:], wy0[:], mod_t[:], op=OP.mult)
                nc.vector.tensor_tensor(wy1[:], fy[:], my1[:], op=OP.mult)
                nc.vector.tensor_tensor(wy1[:], wy1[:], mod_t[:], op=OP.mult)
                nc.vector.tensor_scalar(omf[:], fx_[:], -1.0, 1.0, OP.mult, OP.add)
                nc.vector.tensor_tensor(wx0[:], omf[:], mx0[:], op=OP.mult)
                nc.vector.tensor_tensor(wx1[:], fx_[:], mx1[:], op=OP.mult)

                Wt_g = ctile([LC, 4 * NLC], F32, "Wt_g")
                Wv = Wt_g[:].rearrange("p (r c) -> p r c", r=4)
                nc.vector.tensor_tensor(Wv[:, 0, :], wy0[:], wx0[:], op=OP.mult)
                nc.vector.tensor_tensor(Wv[:, 1, :], wy0[:], wx1[:], op=OP.mult)
                nc.vector.tensor_tensor(Wv[:, 2, :], wy1[:], wx0[:], op=OP.mult)
                nc.vector.tensor_tensor(Wv[:, 3, :], wy1[:], wx1[:], op=OP.mult)

                yc0 = dvt("yc0"); yc1 = dvt("yc1"); xc0 = dvt("xc0"); xc1 = dvt("xc1")
                nc.vector.tensor_scalar(yc0[:], y0s[:], -2.0, 0.0, OP.add, OP.max)
                nc.vector.tensor_scalar(yc0[:], yc0[:], 55.0, 56.0, OP.min, OP.mult)
                nc.vector.tensor_scalar(yc1[:], y0s[:], -1.0, 0.0, OP.add, OP.max)
                nc.vector.tensor_scalar(yc1[:], yc1[:], 55.0, 56.0, OP.min, OP.mult)
                nc.vector.tensor_scalar(xc0[:], x0s[:], -2.0, 0.0, OP.add, OP.max)
                nc.vector.tensor_scalar(xc0[:], xc0[:], 55.0, None, OP.min)
                nc.vector.tensor_scalar(xc1[:], x0s[:], -1.0, 0.0, OP.add, OP.max)
                nc.vector.tensor_scalar(xc1[:], xc1[:], 55.0, None, OP.min)

                If_g = ctile([LC, 4 * NLC], F32R, "If_g")
                Ifv = If_g[:].rearrange("p (r c) -> p r c", r=4)
                nc.vector.tensor_tensor(Ifv[:, 0, :], yc0[:], xc0[:], op=OP.add)
                nc.vector.tensor_tensor(Ifv[:, 1, :], yc0[:], xc1[:], op=OP.add)
                nc.vector.tensor_tensor(Ifv[:, 2, :], yc1[:], xc0[:], op=OP.add)
                nc.vector.tensor_tensor(Ifv[:, 3, :], yc1[:], xc1[:], op=OP.add)

                # --- E: idx wrap on-chip. The gather wants index k (stream
                # order k = (r, c, p112)) at partition k%16, free k//16 =
                # (r, c, j), p112 = 16j + k%16 — and replicated to all 8
                # 16-partition Q7 groups. perm matmul j: out[16b+q, (r c)] =
                # If[16j+q, (r c)] does wrap + replication in one shot.
                pwr = psum.tile([128, 7 * 28], F32, tag="o", bufs=2, name="pwr")
                for j in range(NLC):
                    nc.tensor.matmul(
                        pwr[:, 28 * j:28 * (j + 1)],
                        _r(permr[:, 128 * j:128 * (j + 1)]),
                        _r(If_g[:]),
                        start=True, stop=True,
                    )
                nc.vector.tensor_copy(
                    idxw[g][:].rearrange("p (r c j) -> p r c j", r=4, c=NLC),
                    pwr[:].rearrange("p (j r c) -> p r c j", j=NLC, r=4))

                # wgt: PE transpose -> cheap DRAM roundtrip -> Pool broadcast
                pt = psum.tile([4 * NLC, LC], F32, tag="o", bufs=2, name="pt")
                nc.tensor.transpose(pt[:], Wt_g[:], idn_sb[0:LC, 0:LC])
                WtT = ctile([4 * NLC, LC], F32, "WtT")
                nc.vector.tensor_copy(WtT[:], pt[:])
                nc.sync.dma_start(wgtT_v[g], WtT[:])
                wbc = []
                for r in range(4):
                    wrow_f = ctile([1, L], F32, "wrow_f", bufs=2)
                    nc.sync.dma_start(wrow_f[:], wrow_v[g, r][None, :])
                    t = ctile([128, L], F32, "wbc", bufs=3)
                    nc.gpsimd.partition_broadcast(t[:], wrow_f[:])
                    wbc.append(t)

                # --- F+G: gather + bilinear (scalar_tensor_tensor = 2x DVE) ---
                gat = ctile([128, 4 * L], F32, "gat", bufs=2)
                nc.gpsimd.ap_gather(
                    gat[:], x_sb[g][:].bitcast(F32), idxw[g][:],
                    channels=128, num_elems=HW, d=1, num_idxs=4 * L,
                )
                m01 = ctile([128, L], F32, "m01", bufs=1)
                m23 = ctile([128, L], F32, "m23", bufs=1)
                tmp = ctile([128, L], F32, "biltmp", bufs=1)
                nc.vector.scalar_tensor_tensor(
                    m01[:], gat[:, 0:L], 1.0, wbc[0][:], OP.mult, OP.mult)
                nc.vector.scalar_tensor_tensor(
                    tmp[:], gat[:, L:2 * L], 1.0, wbc[1][:], OP.mult, OP.mult)
                nc.vector.scalar_tensor_tensor(
                    m01[:], m01[:], 1.0, tmp[:], OP.mult, OP.add)
                nc.vector.scalar_tensor_tensor(
                    m23[:], gat[:, 2 * L:3 * L], 1.0, wbc[2][:], OP.mult, OP.mult)
                nc.vector.scalar_tensor_tensor(
                    tmp[:], gat[:, 3 * L:4 * L], 1.0, wbc[3][:], OP.mult, OP.mult)
                nc.vector.scalar_tensor_tensor(
                    m23[:], m23[:], 1.0, tmp[:], OP.mult, OP.add)
                nc.vector.scalar_tensor_tensor(
                    xs_sb[g][:], m01[:], 1.0, m23[:], OP.mult, OP.add)

        dgctx.close()
        xctx.close()   # release x tiles
        if stop_after == "G":
            _dump(xs_sb)
            return

        # ---------------- phase H: k and v^T ----------------
        hpool = ctx.enter_context(tc.tile_pool(name="hpool", bufs=1))
        k_sb = [hpool.tile([128, L], F32R, name=f"k_sb{m}") for m in range(3)]
        for m in range(3):
            for n2 in range(2):
                pk = psum.tile([128, 392], F32, tag="big", bufs=2, name="pk")
                for kc in range(3):
                    nc.tensor.matmul(
                        pk[:],
                        _r(kwk_sb[kc][:, 128 * m:128 * (m + 1)]),
                        _r(xs_sb[kc][:, 392 * n2:392 * (n2 + 1)]),
                        start=(kc == 0), stop=(kc == 2),
                    )
                nc.vector.tensor_copy(k_sb[m][:, 392 * n2:392 * (n2 + 1)], pk[:])

        vTe = [hpool.tile([LC, 6 * 65], F32R, name=f"vTe{lc}") for lc in range(NLC)]
        for lc in range(NLC):
            nc.vector.memset(vTe[lc][:].bitcast(F32), 1.0)
            pv = psum.tile([LC, DIM], F32, tag="big", bufs=2, name="pv")
            for kc in range(3):
                nc.tensor.matmul(
                    pv[:],
                    _r(xs_sb[kc][:, LC * lc:LC * (lc + 1)]),
                    _r(kwv_sb[kc][:, 0:DIM]),
                    start=(kc == 0), stop=(kc == 2),
                )
            dst = vTe[lc][:].rearrange("p (h d) -> p h d", h=6)[:, :, 0:64]
            nc.vector.tensor_copy(dst, pv[:].rearrange("p (h d) -> p h d", h=6))

        if stop_after == "H":
            _dump(k_sb)
            return

        # ---------------- phases I+J: attention + proj, qi-outer ----------------
        with tc.tile_pool(name="opool", bufs=1) as opool, \
             tc.tile_pool(name="apool", bufs=1) as apool:
            O_all = [opool.tile([128, HW], F32R, name=f"O_all{m}") for m in range(3)]

            # Schraudolph fast-exp constants: exp(x) ~= bitcast_f32(round(
            # x * 2^23/ln2 + (127*2^23 - 366393))), max rel err ~3%. Used on
            # the DVE for one of the 7 lc chunks to unload the saturated
            # Activation engine; softmax normalization cancels most of it.
            FEXP_A = 12102203.161561485
            FEXP_B = 1065353216.0 - 366393.0

            def st_part(h, qi, lcs, Es):
                m2, hh = h // 2, h % 2
                for lc in lcs:
                    ps_s = psum.tile([LC, QC], F32, tag="s", bufs=4, name="ps_s")
                    nc.tensor.matmul(
                        ps_s[:],
                        _r(k_sb[m2][64 * hh:64 * hh + 64, LC * lc:LC * (lc + 1)]),
                        _r(q_sb[m2][64 * hh:64 * hh + 64, QC * qi:QC * (qi + 1)]),
                        start=True, stop=True,
                    )
                    if lc >= NLC - 1:
                        f = apool.tile([LC, QC], F32, tag="fe", bufs=2, name="fe")
                        nc.vector.tensor_scalar(f[:], ps_s[:], FEXP_A, FEXP_B,
                                                OP.mult, OP.add)
                        Ei = apool.tile([LC, QC], I32, tag="E6i", bufs=2, name="E6i")
                        nc.vector.tensor_copy(Ei[:], f[:])
                        # value-preserving f32r rounding pass (verifier requires
                        # matmul operands to be produced rounded-to-f32r)
                        E = apool.tile([LC, QC], F32R, tag="E6", bufs=4, name="E6")
                        nc.vector.tensor_copy(E[:], Ei[:].bitcast(F32))
                        Es.append(E[:])
                    else:
                        E = apool.tile([LC, QC], F32R, tag="E", bufs=16, name="E")
                        nc.scalar.activation(E[:], ps_s[:], AF.Exp)
                        Es.append(E[:])

            def ot_mm(h, qi, Es):
                ps_o = psum.tile([128, QC], F32, tag="o", bufs=2, name="ps_o")
                for lc in range(NLC):
                    nc.tensor.matmul(
                        ps_o[0:65, :],
                        _r(vTe[lc][:, 65 * h:65 * (h + 1)]),
                        _r(Es[lc]),
                        start=(lc == 0), stop=(lc == NLC - 1),
                    )
                return ps_o

            def ot_norm(h, qi, ps_o):
                m2, hh = h // 2, h % 2
                rec = apool.tile([1, QC], F32, tag="rec", bufs=4, name="rec")
                with nc.allow_low_precision(reason="fp32-width"):
                    nc.vector.reciprocal(rec[:], ps_o[64:65, :])
                rbc = apool.tile([64, QC], F32, tag="rbc", bufs=4, name="rbc")
                nc.gpsimd.partition_broadcast(rbc[:], rec[:])
                oslice = O_all[m2][64 * hh:64 * hh + 64, QC * qi:QC * (qi + 1)]
                nc.vector.tensor_tensor(oslice, ps_o[0:64, :], rbc[:], op=OP.mult)

            def proj_phase(qi):
                for m in range(3):
                    pp = psum.tile([128, QC], F32, tag="o", bufs=2, name="pp")
                    for kc in range(3):
                        nc.tensor.matmul(
                            pp[:],
                            _r(pjw_sb[kc][:, 128 * m:128 * (m + 1)]),
                            _r(O_all[kc][:, QC * qi:QC * (qi + 1)]),
                            start=(kc == 0), stop=(kc == 2),
                        )
                    y = apool.tile([128, QC], F32, tag="y", bufs=3, name="y")
                    nc.vector.tensor_scalar(y[:], pp[:], pjb_sb[:, m:m + 1], None,
                                            OP.add)
                    nc.sync.dma_start(
                        out_d[128 * m:128 * (m + 1), QC * qi:QC * (qi + 1)], y[:])

            # qi outer so each q-chunk's proj can start as soon as its 6 heads
            # are done. Per step the PE stream is S(it)[0:4] | O(it-1) |
            # S(it)[4:7]: the O matmuls keep PE busy while exp(it, lc0) drains,
            # so the s-psum ring (4 bufs < 7 lc) never head-of-line blocks.
            attn_iters = [(h, qi) for qi in range(NQC) for h in range(NUM_HEAD)]
            if stop_after.startswith("I1"):
                attn_iters = attn_iters[:1]
            pending = None
            for it in attn_iters:
                Es = []
                st_part(it[0], it[1], range(0, 4), Es)
                if pending is not None:
                    ps_o = ot_mm(pending[0][0], pending[0][1], pending[1])
                st_part(it[0], it[1], range(4, NLC), Es)
                if pending is not None:
                    ot_norm(pending[0][0], pending[0][1], ps_o)
                    if pending[0][0] == NUM_HEAD - 1 and not stop_after:
                        proj_phase(pending[0][1])
                pending = (it, Es)
            if pending is not None:
                ps_o = ot_mm(pending[0][0], pending[0][1], pending[1])
                ot_norm(pending[0][0], pending[0][1], ps_o)
                if not stop_after:
                    proj_phase(pending[0][1])

            if stop_after.startswith("I1") or stop_after == "I":
                _dump(O_all[:1] if stop_after.startswith("I1") else O_all)
                return


def host_prep(inputs):
    """Shared (per-core-identical) weight prep. Returns dict of np arrays."""
    f = np.float32
    q_w = np.asarray(inputs["q_w"], f)
    kv_w = np.asarray(inputs["kv_w"], f)
    proj_w = np.asarray(inputs["proj_w"], f)
    proj_b = np.asarray(inputs["proj_b"], f)
    dw_w = np.asarray(inputs["dw_w"], f)
    dw_b = np.asarray(inputs["dw_b"], f)
    bn_w = np.asarray(inputs["bn_w"], f)
    bn_b = np.asarray(inputs["bn_b"], f)
    bn_mean = np.asarray(inputs["bn_mean"], f)
    bn_var = np.asarray(inputs["bn_var"], f)
    pw_w = np.asarray(inputs["pw_w"], f)

    bn_s = (bn_w / np.sqrt(bn_var + BN_EPS)).astype(f)
    bn_t = ((dw_b - bn_mean) * bn_s + bn_b).astype(f)

    # wrap permutation: perm[p, 128j + 16b + q] = 1 iff p == 16j + q
    perm = np.zeros((LC, NLC, 8, 16), f)
    for j in range(NLC):
        for q in range(16):
            perm[16 * j + q, j, :, q] = 1.0
    perm = perm.reshape(LC, NLC * 128)

    p = np.arange(LC)
    c = np.arange(NLC)
    ytab_col = (4 * c[None, :] + p[:, None] // 28 + 0.5 + 2.0 / A).astype(f)  # [112, 7]
    ytab = np.tile(ytab_col, (1, G))                                          # [112, 21]
    xtab_col = (p % 28 + 0.5 + 2.0 / A).astype(f)[:, None]
    xtab = np.tile(xtab_col, (1, G * NLC))

    return {
        "qw_t": np.ascontiguousarray(q_w.T),
        "kwk_t": np.ascontiguousarray((kv_w[:DIM] * SCALE).T),
        "kwv_t": np.ascontiguousarray(kv_w[DIM:].T),
        "pw_t": np.ascontiguousarray(pw_w.T),
        "projw_t": np.ascontiguousarray(proj_w.T),
        "projb_rs": np.ascontiguousarray(proj_b.reshape(3, NGD).T),
        "dww": np.ascontiguousarray(dw_w.reshape(NGD, 25)),
        "bn_s": bn_s.reshape(NGD, 1),
        "bn_t": bn_t.reshape(NGD, 1),
        "ident": np.eye(128, dtype=f),
        "perm": perm,
        "ytab": ytab,
        "xtab": xtab,
    }


_NC_CACHE = {}


def _get_nc(gelu_exact=True):
    key = bool(gelu_exact)
    if key not in _NC_CACHE:
        _NC_CACHE[key] = build_nc(gelu_exact=key)
    return _NC_CACHE[key]


def make_in_maps(inputs):
    shared = host_prep(inputs)
    x = np.asarray(inputs["x"], np.float32)
    in_maps = []
    for i in range(B):
        m = dict(shared)
        m["x"] = np.ascontiguousarray(x[i].reshape(DIM, HW))
        in_maps.append(m)
    return in_maps


def run_spmd(inputs, trace=False):
    """Run on the 8 NeuronCores; returns (out (8,384,56,56), BassKernelResults)."""
    nc = _get_nc(True)
    in_maps = make_in_maps(inputs)
    res = bass_utils.run_bass_kernel_spmd(
        nc, in_maps, core_ids=list(range(B)), trace=trace,
    )
    out = np.stack([r["out"].reshape(DIM, H, W) for r in res.results], axis=0)
    return out, res


def kernel(**inputs) -> np.ndarray:
    out, _ = run_spmd(inputs, trace=False)
    return out


# revision 47
# speedup vs baseline: 1.4233x; 1.0078x over previous
"""Deformable multi-head sparse attention (DMSA) Bass kernel for Trainium2.

Contract: kernel(**inputs) takes the FULL unsharded inputs (as produced by
setup_inputs()) and returns the FULL output (B, 384, 56, 56) float32.
Internally shards batch B=8 across 8 NeuronCores (pure data parallel,
no collectives), one batch element per core.

Self-contained: hardcodes all shapes; does not read any sibling files.
"""
import sys

for _p in ("/opt/trn_rl_repo", "/opt/pypackages"):
    if _p not in sys.path:
        sys.path.insert(0, _p)

import numpy as np

import concourse.bass as bass
import concourse.mybir as mybir
import concourse.tile as tile
from concourse import bacc
from concourse import bass_utils

F32 = mybir.dt.float32
F32R = mybir.dt.float32r
BF16 = mybir.dt.bfloat16
I16 = mybir.dt.int16
I32 = mybir.dt.int32
AF = mybir.ActivationFunctionType
OP = mybir.AluOpType

# problem constants
B = 8
DIM = 384
DIM_HEAD = 64
NUM_HEAD = 6
G = 3            # deformable groups
NGD = 128        # channels per group
H = 56
W = 56
HW = H * W       # 3136
HO = 28
WO = 28
L = HO * WO      # 784
SCALE = DIM_HEAD ** -0.5
BN_EPS = 1e-6
A = (W - 1) / WO   # 55/28, same for y since H==W and HO==WO
PADD = 60          # padded dwconv input edge (56 + 2*2)

QC = 448           # q-position chunk (free dim of attention matmuls)
NQC = HW // QC     # 7
LC = 112           # kv-position chunk (partition dim of S^T)
NLC = L // LC      # 7


def _r(ap):
    return ap


def build_nc(gelu_exact: bool = True, stop_after: str = ""):
    """Build the per-core Bass program (SPMD: same NEFF on all 8 cores)."""
    nc = bacc.Bacc("TRN2", target_bir_lowering=False, debug=False, num_devices=B)

    din = {}
    def dt_in(name, shape, dtype=F32):
        din[name] = nc.dram_tensor(name, shape, dtype, kind="ExternalInput").ap()
        return din[name]

    dt_in("x", [DIM, HW])
    dt_in("qw_t", [DIM, DIM])
    dt_in("kwk_t", [DIM, DIM])
    dt_in("kwv_t", [DIM, DIM])
    dt_in("pw_t", [NGD, 3])
    dt_in("projw_t", [DIM, DIM])
    dt_in("projb_rs", [NGD, 3])
    dt_in("dww", [NGD, 25])
    dt_in("bn_s", [NGD, 1])
    dt_in("bn_t", [NGD, 1])
    dt_in("ident", [128, 128])
    dt_in("perm", [LC, 7 * 128])
    dt_in("ytab", [LC, 21])
    dt_in("xtab", [LC, 21])

    out_d = nc.dram_tensor("out", [DIM, HW], F32, kind="ExternalOutput").ap()

    with tile.TileContext(nc) as tc:
        _body(nc, tc, din, out_d, gelu_exact, stop_after)

    nc.compile()
    return nc


def _body(nc, tc, din, out_d, gelu_exact, stop_after=""):
    import contextlib
    ctx = contextlib.ExitStack()
    with ctx:
        # persistent pools (whole kernel)
        wpool = ctx.enter_context(tc.tile_pool(name="wpool", bufs=1))
        spool = ctx.enter_context(tc.tile_pool(name="spool", bufs=1))
        qpool = ctx.enter_context(tc.tile_pool(name="qpool", bufs=1))
        psum = ctx.enter_context(tc.tile_pool(name="psum", bufs=1, space="PSUM"))
        dram = ctx.enter_context(tc.tile_pool(name="dram", bufs=1, space="DRAM"))

        # ---------------- phase A: weight loads ----------------
        # fp32r matmul operands must be produced by compute ops (DMA does not
        # round to fp32r), so every DMA-loaded matmul operand goes through a
        # conversion copy into an F32R tile.
        def load_small(key, shape, dtype=F32):
            t = spool.tile(shape, dtype, name=key + "_sb")
            nc.scalar.dma_start(t[:], din[key][:])
            return t

        pjb_sb = load_small("projb_rs", [NGD, 3])
        dww_sb = load_small("dww", [NGD, 25])
        bns_sb = load_small("bn_s", [NGD, 1])
        bnt_sb = load_small("bn_t", [NGD, 1])
        idn_sb = load_small("ident", [128, 128])
        ytab_sb = load_small("ytab", [LC, 21])
        xtab_sb = load_small("xtab", [LC, 21])

        # x pool: released after the gather phase
        xctx = contextlib.ExitStack()
        xpool = xctx.enter_context(tc.tile_pool(name="xpool", bufs=1))
        x_sb = [xpool.tile([128, HW], F32R, name=f"x_sb{g}") for g in range(G)]
        qw_sb, kwk_sb, kwv_sb, pjw_sb = [], [], [], []
        with tc.tile_pool(name="ldpool", bufs=1) as ldpool:
            # x first on the SP DMA queue (it gates the q matmuls); the
            # non-q weights go via the Activation DMA queue in parallel
            for g in range(G):
                xt = ldpool.tile([128, HW], F32, tag="xtmp", bufs=3, name="xt")
                nc.sync.dma_start(xt[:], din["x"][128 * g:128 * (g + 1), :])
                if g % 2 == 0:
                    nc.scalar.activation(x_sb[g][:], xt[:], AF.Copy)
                else:
                    nc.vector.tensor_copy(x_sb[g][:], xt[:])
            for name, key, dst in (("qw", "qw_t", qw_sb), ("kwk", "kwk_t", kwk_sb),
                                   ("kwv", "kwv_t", kwv_sb), ("pjw", "projw_t", pjw_sb)):
                for kc in range(3):
                    wt = ldpool.tile([128, DIM], F32, tag="wtmp", bufs=4, name="wt")
                    nc.scalar.dma_start(wt[:], din[key][128 * kc:128 * (kc + 1), :])
                    t = wpool.tile([128, DIM], F32R, name=f"{name}_r{kc}")
                    nc.vector.tensor_copy(t[:], wt[:])
                    dst.append(t)
            pw_sb = spool.tile([NGD, 3], F32, name="pw_sb")
            nc.scalar.dma_start(pw_sb[:], din["pw_t"][:])
            # idx-wrap permutation matrices (7x [112, 128]), used as f32r lhsT
            pm = ldpool.tile([LC, 7 * 128], F32, tag="pm", bufs=1, name="pm")
            nc.scalar.dma_start(pm[:], din["perm"][:])
            permr = wpool.tile([LC, 7 * 128], F32R, name="permr")
            nc.vector.tensor_copy(permr[:], pm[:])

        # ---------------- phase B: q = q_w @ x ----------------
        q_sb = [qpool.tile([128, HW], F32R, name=f"q_sb{m}") for m in range(3)]
        for m in range(3):
            for n in range(NQC):
                pq = psum.tile([128, QC], F32, tag="o", bufs=2, name="pq")
                for kc in range(3):
                    nc.tensor.matmul(
                        pq[:],
                        _r(qw_sb[kc][:, 128 * m:128 * (m + 1)]),
                        _r(x_sb[kc][:, QC * n:QC * (n + 1)]),
                        start=(kc == 0), stop=(kc == 2),
                    )
                # spread psum evacuation across engines (gpsimd has no PSUM port)
                dstap = q_sb[m][:, QC * n:QC * (n + 1)]
                if n % 2 == 0:
                    nc.vector.tensor_copy(dstap, pq[:])
                else:
                    nc.scalar.activation(dstap, pq[:], AF.Copy)

        def _dump(tiles):
            for mm, tt in enumerate(tiles[:3]):
                nc.sync.dma_start(out_d[128 * mm:128 * (mm + 1), 0:tt.shape[1]],
                                  tt[:].bitcast(F32) if tt.dtype != F32 else tt[:])

        if stop_after == "B":
            _dump(q_sb)
            xctx.close()
            return

        # ---------------- phases C..G: per-group pipelined ----------------
        # wgt DRAM layout: flat (g, r, c, p); written from the PE-transposed
        # [28, 112] tile so both the write (28 runs of 448B) and the per-row
        # reads (contiguous 784) stay descriptor-cheap.
        wgt_dr = dram.tile([G * 4 * NLC * LC], F32)
        wgtT_v = wgt_dr.rearrange("(g rc p) -> g rc p", g=G, rc=4 * NLC)
        wrow_v = wgt_dr.rearrange("(g r n) -> g r n", g=G, r=4)

        xs_sb = [qpool.tile([128, L], F32R, name=f"xs_sb{g}") for g in range(G)]
        idxw = [spool.tile([128, 196], I16, name=f"idxw{g}") for g in range(G)]

        dgctx = contextlib.ExitStack()
        dgpool = dgctx.enter_context(tc.tile_pool(name="dgpool", bufs=1))
        diag = dgpool.tile([128, 25 * 128], BF16, name="diag")
        for t in range(25):
            # diag[:, t] = ident * dww[:, t]  (per-partition scalar broadcast)
            nc.gpsimd.tensor_scalar(
                diag[:, 128 * t:128 * (t + 1)], idn_sb[:],
                dww_sb[:, t:t + 1], None, OP.mult,
            )

        with tc.tile_pool(name="cpool", bufs=1) as cpool:
            def ctile(shape, dtype, tag, bufs=2):
                return cpool.tile(shape, dtype, tag=tag, bufs=bufs, name=tag)

            # pads for all groups up front (bf16: dwconv runs at the same
            # PE rate, half the SBUF) so the Act queue never head-of-line
            # blocks a later group's pad copy behind an earlier gelu
            pads = []
            for g in range(G):
                pad = ctile([128, PADD * PADD], BF16, "pad", bufs=3)
                pad_v = pad[:].rearrange("p (h w) -> p h w", w=PADD)
                nc.gpsimd.memset(pad_v[:, 0:2, :], 0.0)
                nc.gpsimd.memset(pad_v[:, 58:60, :], 0.0)
                nc.gpsimd.memset(pad_v[:, 2:58, 0:2], 0.0)
                nc.gpsimd.memset(pad_v[:, 2:58, 58:60], 0.0)
                qv = q_sb[g][:].rearrange("p (h w) -> p h w", w=W)
                nc.scalar.activation(pad_v[:, 2:58, 2:58], qv[:], AF.Copy)
                pads.append(pad_v)

            for g in range(G):
                pad_v = pads[g]
                # --- C3+C4: depthwise conv + BN + GELU ---
                gelu = ctile([128, L], F32, "gelu", bufs=2)
                for nn in range(2):
                    pdw = psum.tile([128, 392], F32, tag="o", bufs=2, name="pdw")
                    for t in range(25):
                        ty, tx = t // 5, t % 5
                        rhs = pad_v[:, ty + 28 * nn: ty + 28 * nn + 28: 2, tx: tx + 56: 2]
                        nc.tensor.matmul(
                            pdw[:], _r(diag[:, 128 * t:128 * (t + 1)]), _r(rhs),
                            start=(t == 0), stop=(t == 24),
                        )
                    gout = gelu[:, 392 * nn:392 * (nn + 1)]
                    nc.scalar.activation(gout, pdw[:], AF.Gelu,
                                         bias=bnt_sb[:, 0:1], scale=bns_sb[:, 0:1])

                # --- C5: om^T = gelu^T @ pw -> [112 pos, (chunk, ch)] ---
                pom = psum.tile([LC, 21], F32, tag="o", bufs=2, name="pom")
                for c in range(NLC):
                    nc.tensor.matmul(
                        pom[:, 3 * c:3 * (c + 1)],
                        gelu[:, LC * c:LC * (c + 1)],
                        pw_sb[:, 0:3],
                        start=True, stop=True,
                    )
                om_g = ctile([LC, 21], F32, "om_g")
                nc.scalar.activation(om_g[:], pom[:], AF.Copy)

                # --- D: position math on [112, 7] slices ---
                om_v = om_g[:].rearrange("p (k ch) -> p k ch", ch=3)
                om0, om1, om2 = om_v[:, :, 0], om_v[:, :, 1], om_v[:, :, 2]
                yt = ytab_sb[:, 0:NLC]
                xt = xtab_sb[:, 0:NLC]

                def dvt(tag):
                    return ctile([LC, NLC], F32, tag)

                # sigmoid(x) = 0.5*tanh(0.5x) + 0.5 (tanh shares the gelu/exp
                # act tables; sigmoid would force table reloads)
                ty_t = dvt("ty_t"); tx_t = dvt("tx_t"); mod_t = dvt("mod_t")
                nc.scalar.activation(ty_t[:], om0, AF.Tanh)
                nc.scalar.activation(tx_t[:], om1, AF.Tanh)
                sg_t = dvt("sg_t")
                nc.scalar.activation(sg_t[:], om2, AF.Tanh, scale=0.5)
                nc.vector.tensor_scalar(sg_t[:], sg_t[:], 0.5, 0.5, OP.mult, OP.add)
                nc.scalar.activation(mod_t[:], sg_t[:], AF.Tanh, scale=0.5)
                nc.vector.tensor_scalar(mod_t[:], mod_t[:], 0.5, 0.5, OP.mult, OP.add)

                # ytab/xtab are pre-scaled by A on the host: one fused op each.
                # y-path on DVE, x-path on gpsimd — the two chains are
                # independent, halving the serial position-math latency.
                gy2 = dvt("gy2"); gx2 = dvt("gx2")
                nc.vector.scalar_tensor_tensor(
                    gy2[:], ty_t[:], float(A), yt, OP.mult, OP.add)
                nc.gpsimd.tensor_scalar(gx2[:], tx_t[:], float(A), None, OP.mult)
                nc.gpsimd.tensor_tensor(gx2[:], gx2[:], xt, op=OP.add)

                def floor_of(eng, gt, tag):
                    ii = ctile([LC, NLC], I32, tag + "_i")
                    eng.tensor_copy(ii[:], gt[:])
                    ff = dvt(tag + "_f")
                    eng.tensor_copy(ff[:], ii[:])
                    fxm = dvt(tag + "_fix")
                    eng.tensor_tensor(fxm[:], ff[:], gt[:], op=OP.is_gt)
                    eng.tensor_tensor(ff[:], ff[:], fxm[:], op=OP.subtract)
                    return ff

                y0s = floor_of(nc.vector, gy2, "y0s")
                x0s = floor_of(nc.vector, gx2, "x0s")

                fy = dvt("fy"); fx_ = dvt("fx_")
                nc.vector.tensor_tensor(fy[:], gy2[:], y0s[:], op=OP.subtract)
                nc.gpsimd.tensor_tensor(fx_[:], gx2[:], x0s[:], op=OP.subtract)

                my0 = dvt("my0"); my1 = dvt("my1"); mx0 = dvt("mx0"); mx1 = dvt("mx1")
                nc.vector.tensor_scalar(my0[:], gy2[:], 2.0, None, OP.is_ge)
                nc.vector.tensor_scalar(my1[:], gy2[:], 57.0, None, OP.is_lt)
                nc.vector.tensor_scalar(mx0[:], gx2[:], 2.0, None, OP.is_ge)
                nc.vector.tensor_scalar(mx1[:], gx2[:], 57.0, None, OP.is_lt)

                wy0 = dvt("wy0"); wy1 = dvt("wy1"); wx0 = dvt("wx0"); wx1 = dvt("wx1")
                omf = dvt("omf"); omfx = dvt("omfx")
                nc.vector.tensor_scalar(omf[:], fy[:], -1.0, 1.0, OP.mult, OP.add)
                nc.vector.tensor_tensor(wy0[:], omf[:], my0[:], op=OP.mult)
                nc.vector.tensor_tensor(wy0[:], wy0[:], mod_t[:], op=OP.mult)
                nc.vector.tensor_tensor(wy1[:], fy[:], my1[:], op=OP.mult)
                nc.vector.tensor_tensor(wy1[:], wy1[:], mod_t[:], op=OP.mult)
                nc.gpsimd.tensor_scalar(omfx[:], fx_[:], -1.0, 1.0, OP.mult, OP.add)
                nc.gpsimd.tensor_tensor(wx0[:], omfx[:], mx0[:], op=OP.mult)
                nc.gpsimd.tensor_tensor(wx1[:], fx_[:], mx1[:], op=OP.mult)

                Wt_g = ctile([LC, 4 * NLC], F32, "Wt_g")
                Wv = Wt_g[:].rearrange("p (r c) -> p r c", r=4)
                nc.vector.tensor_tensor(Wv[:, 0, :], wy0[:], wx0[:], op=OP.mult)
                nc.vector.tensor_tensor(Wv[:, 1, :], wy0[:], wx1[:], op=OP.mult)
                nc.vector.tensor_tensor(Wv[:, 2, :], wy1[:], wx0[:], op=OP.mult)
                nc.vector.tensor_tensor(Wv[:, 3, :], wy1[:], wx1[:], op=OP.mult)

                yc0 = dvt("yc0"); yc1 = dvt("yc1"); xc0 = dvt("xc0"); xc1 = dvt("xc1")
                nc.vector.tensor_scalar(yc0[:], y0s[:], -2.0, 0.0, OP.add, OP.max)
                nc.vector.tensor_scalar(yc0[:], yc0[:], 55.0, 56.0, OP.min, OP.mult)
                nc.vector.tensor_scalar(yc1[:], y0s[:], -1.0, 0.0, OP.add, OP.max)
                nc.vector.tensor_scalar(yc1[:], yc1[:], 55.0, 56.0, OP.min, OP.mult)
                nc.gpsimd.tensor_scalar(xc0[:], x0s[:], -2.0, 0.0, OP.add, OP.max)
                nc.gpsimd.tensor_scalar(xc0[:], xc0[:], 55.0, None, OP.min)
                nc.gpsimd.tensor_scalar(xc1[:], x0s[:], -1.0, 0.0, OP.add, OP.max)
                nc.gpsimd.tensor_scalar(xc1[:], xc1[:], 55.0, None, OP.min)

                If_g = ctile([LC, 4 * NLC], F32R, "If_g")
                Ifv = If_g[:].rearrange("p (r c) -> p r c", r=4)
                nc.vector.tensor_tensor(Ifv[:, 0, :], yc0[:], xc0[:], op=OP.add)
                nc.vector.tensor_tensor(Ifv[:, 1, :], yc0[:], xc1[:], op=OP.add)
                nc.vector.tensor_tensor(Ifv[:, 2, :], yc1[:], xc0[:], op=OP.add)
                nc.vector.tensor_tensor(Ifv[:, 3, :], yc1[:], xc1[:], op=OP.add)

                # --- E: idx wrap on-chip. The gather wants index k (stream
                # order k = (r, c, p112)) at partition k%16, free k//16 =
                # (r, c, j), p112 = 16j + k%16 — and replicated to all 8
                # 16-partition Q7 groups. perm matmul j: out[16b+q, (r c)] =
                # If[16j+q, (r c)] does wrap + replication in one shot.
                pwr = psum.tile([128, 7 * 28], F32, tag="o", bufs=2, name="pwr")
                for j in range(NLC):
                    nc.tensor.matmul(
                        pwr[:, 28 * j:28 * (j + 1)],
                        _r(permr[:, 128 * j:128 * (j + 1)]),
                        _r(If_g[:]),
                        start=True, stop=True,
                    )
                nc.vector.tensor_copy(
                    idxw[g][:].rearrange("p (r c j) -> p r c j", r=4, c=NLC),
                    pwr[:].rearrange("p (j r c) -> p r c j", j=NLC, r=4))

                # wgt: PE transpose -> cheap DRAM roundtrip -> Pool broadcast
                pt = psum.tile([4 * NLC, LC], F32, tag="o", bufs=2, name="pt")
                nc.tensor.transpose(pt[:], Wt_g[:], idn_sb[0:LC, 0:LC])
                WtT = ctile([4 * NLC, LC], F32, "WtT")
                nc.vector.tensor_copy(WtT[:], pt[:])
                nc.sync.dma_start(wgtT_v[g], WtT[:])
                wbc = []
                for r in range(4):
                    wrow_f = ctile([1, L], F32, "wrow_f", bufs=4)
                    nc.sync.dma_start(wrow_f[:], wrow_v[g, r][None, :])
                    t = ctile([128, L], F32, "wbc", bufs=4)
                    nc.gpsimd.partition_broadcast(t[:], wrow_f[:])
                    wbc.append(t)

                # --- F+G: gather + bilinear (scalar_tensor_tensor = 2x DVE) ---
                gat = ctile([128, 4 * L], F32, "gat", bufs=2)
                nc.gpsimd.ap_gather(
                    gat[:], x_sb[g][:].bitcast(F32), idxw[g][:],
                    channels=128, num_elems=HW, d=1, num_idxs=4 * L,
                )
                m01 = ctile([128, L], F32, "m01", bufs=1)
                m23 = ctile([128, L], F32, "m23", bufs=1)
                tmp = ctile([128, L], F32, "biltmp", bufs=1)
                nc.vector.scalar_tensor_tensor(
                    m01[:], gat[:, 0:L], 1.0, wbc[0][:], OP.mult, OP.mult)
                nc.vector.scalar_tensor_tensor(
                    tmp[:], gat[:, L:2 * L], 1.0, wbc[1][:], OP.mult, OP.mult)
                nc.vector.scalar_tensor_tensor(
                    m01[:], m01[:], 1.0, tmp[:], OP.mult, OP.add)
                nc.vector.scalar_tensor_tensor(
                    m23[:], gat[:, 2 * L:3 * L], 1.0, wbc[2][:], OP.mult, OP.mult)
                nc.vector.scalar_tensor_tensor(
                    tmp[:], gat[:, 3 * L:4 * L], 1.0, wbc[3][:], OP.mult, OP.mult)
                nc.vector.scalar_tensor_tensor(
                    m23[:], m23[:], 1.0, tmp[:], OP.mult, OP.add)
                nc.vector.scalar_tensor_tensor(
                    xs_sb[g][:], m01[:], 1.0, m23[:], OP.mult, OP.add)

        dgctx.close()
        xctx.close()   # release x tiles
        if stop_after == "G":
            _dump(xs_sb)
            return

        # ---------------- phase H: k and v^T ----------------
        hpool = ctx.enter_context(tc.tile_pool(name="hpool", bufs=1))
        k_sb = [hpool.tile([128, L], F32R, name=f"k_sb{m}") for m in range(3)]
        for m in range(3):
            for n2 in range(2):
                pk = psum.tile([128, 392], F32, tag="o", bufs=2, name="pk")
                for kc in range(3):
                    nc.tensor.matmul(
                        pk[:],
                        _r(kwk_sb[kc][:, 128 * m:128 * (m + 1)]),
                        _r(xs_sb[kc][:, 392 * n2:392 * (n2 + 1)]),
                        start=(kc == 0), stop=(kc == 2),
                    )
                nc.scalar.activation(k_sb[m][:, 392 * n2:392 * (n2 + 1)], pk[:],
                                     AF.Copy)

        vTe = [hpool.tile([LC, 6 * 65], F32R, name=f"vTe{lc}") for lc in range(NLC)]
        for lc in range(NLC):
            nc.gpsimd.memset(vTe[lc][:].bitcast(F32), 1.0)
            pv = psum.tile([LC, DIM], F32, tag="o", bufs=2, name="pv")
            for kc in range(3):
                nc.tensor.matmul(
                    pv[:],
                    _r(xs_sb[kc][:, LC * lc:LC * (lc + 1)]),
                    _r(kwv_sb[kc][:, 0:DIM]),
                    start=(kc == 0), stop=(kc == 2),
                )
            dst = vTe[lc][:].rearrange("p (h d) -> p h d", h=6)[:, :, 0:64]
            nc.vector.tensor_copy(dst, pv[:].rearrange("p (h d) -> p h d", h=6))

        if stop_after == "H":
            _dump(k_sb)
            return

        # ---------------- phases I+J: attention + proj, qi-outer ----------------
        with tc.tile_pool(name="opool", bufs=1) as opool, \
             tc.tile_pool(name="apool", bufs=1) as apool:
            O_all = [opool.tile([128, HW], F32R, name=f"O_all{m}") for m in range(3)]

            # Schraudolph fast-exp constants: exp(x) ~= bitcast_f32(round(
            # x * 2^23/ln2 + (127*2^23 - 366393))), max rel err ~3%. Used on
            # the DVE for one of the 7 lc chunks to unload the saturated
            # Activation engine; softmax normalization cancels most of it.
            FEXP_A = 12102203.161561485
            FEXP_B = 1065353216.0 - 366393.0

            def st_pairs(h, qi, pairs, Es):
                # S matmuls for lc pairs into a 2-bank psum tile (448-elem
                # halves at 0 and 512 so neither crosses a bank), one Exp per
                # pair: halves the Activation instruction count.
                m2, hh = h // 2, h % 2
                kv = k_sb[m2][64 * hh:64 * hh + 64, :]
                qv = q_sb[m2][64 * hh:64 * hh + 64, QC * qi:QC * (qi + 1)]
                for lc0 in pairs:
                    ps_s = psum.tile([LC, 1024], F32, tag="s", bufs=2, name="ps_s")
                    for half, lc in ((0, lc0), (1, lc0 + 1)):
                        nc.tensor.matmul(
                            ps_s[:, 512 * half:512 * half + QC],
                            _r(kv[:, LC * lc:LC * (lc + 1)]),
                            _r(qv),
                            start=True, stop=True,
                        )
                    E = apool.tile([LC, 2 * QC], F32R, tag="E", bufs=8, name="E")
                    nc.scalar.activation(
                        E[:].rearrange("p (two x) -> p two x", x=QC),
                        ps_s[:].rearrange("p (two x) -> p two x", x=512)[:, :, 0:QC],
                        AF.Exp)
                    Es.append(E[:, 0:QC])
                    Es.append(E[:, QC:2 * QC])

            def st_fexp(h, qi, Es):
                # lc6 on the DVE via Schraudolph fast-exp
                m2, hh = h // 2, h % 2
                lc = NLC - 1
                ps_s = psum.tile([LC, QC], F32, tag="s1", bufs=2, name="ps_s1")
                nc.tensor.matmul(
                    ps_s[:],
                    _r(k_sb[m2][64 * hh:64 * hh + 64, LC * lc:LC * (lc + 1)]),
                    _r(q_sb[m2][64 * hh:64 * hh + 64, QC * qi:QC * (qi + 1)]),
                    start=True, stop=True,
                )
                f = apool.tile([LC, QC], F32, tag="fe", bufs=2, name="fe")
                nc.vector.tensor_scalar(f[:], ps_s[:], FEXP_A, FEXP_B,
                                        OP.mult, OP.add)
                Ei = apool.tile([LC, QC], I32, tag="E6i", bufs=2, name="E6i")
                nc.vector.tensor_copy(Ei[:], f[:])
                # value-preserving f32r rounding pass (verifier requires
                # matmul operands to be produced rounded-to-f32r)
                E = apool.tile([LC, QC], F32R, tag="E6", bufs=4, name="E6")
                nc.vector.tensor_copy(E[:], Ei[:].bitcast(F32))
                Es.append(E[:])

            def ot_mm(h, qi, Es):
                ps_o = psum.tile([128, QC], F32, tag="o", bufs=2, name="ps_o")
                for lc in range(NLC):
                    nc.tensor.matmul(
                        ps_o[0:65, :],
                        _r(vTe[lc][:, 65 * h:65 * (h + 1)]),
                        _r(Es[lc]),
                        start=(lc == 0), stop=(lc == NLC - 1),
                    )
                return ps_o

            def ot_norm(h, qi, ps_o):
                m2, hh = h // 2, h % 2
                rec = apool.tile([1, QC], F32, tag="rec", bufs=4, name="rec")
                with nc.allow_low_precision(reason="fp32-width"):
                    nc.vector.reciprocal(rec[:], ps_o[64:65, :])
                rbc = apool.tile([64, QC], F32, tag="rbc", bufs=4, name="rbc")
                nc.gpsimd.partition_broadcast(rbc[:], rec[:])
                oslice = O_all[m2][64 * hh:64 * hh + 64, QC * qi:QC * (qi + 1)]
                nc.vector.tensor_tensor(oslice, ps_o[0:64, :], rbc[:], op=OP.mult)

            def proj_phase(qi):
                for m in range(3):
                    pp = psum.tile([128, QC], F32, tag="o", bufs=2, name="pp")
                    for kc in range(3):
                        nc.tensor.matmul(
                            pp[:],
                            _r(pjw_sb[kc][:, 128 * m:128 * (m + 1)]),
                            _r(O_all[kc][:, QC * qi:QC * (qi + 1)]),
                            start=(kc == 0), stop=(kc == 2),
                        )
                    y = apool.tile([128, QC], F32, tag="y", bufs=3, name="y")
                    nc.vector.tensor_scalar(y[:], pp[:], pjb_sb[:, m:m + 1], None,
                                            OP.add)
                    nc.sync.dma_start(
                        out_d[128 * m:128 * (m + 1), QC * qi:QC * (qi + 1)], y[:])

            # qi outer so each q-chunk's proj can start as soon as its 6 heads
            # are done. Per step the PE stream is S(it)[0:4] | O(it-1) |
            # S(it)[4:7]: the O matmuls keep PE busy while exp(it, lc0) drains,
            # so the s-psum ring (4 bufs < 7 lc) never head-of-line blocks.
            attn_iters = [(h, qi) for qi in range(NQC) for h in range(NUM_HEAD)]
            if stop_after.startswith("I1"):
                attn_iters = attn_iters[:1]
            pending = None
            for it in attn_iters:
                Es = []
                st_pairs(it[0], it[1], (0, 2), Es)
                if pending is not None:
                    ps_o = ot_mm(pending[0][0], pending[0][1], pending[1])
                st_pairs(it[0], it[1], (4,), Es)
                st_fexp(it[0], it[1], Es)
                if pending is not None:
                    ot_norm(pending[0][0], pending[0][1], ps_o)
                    if pending[0][0] == NUM_HEAD - 1 and not stop_after:
                        proj_phase(pending[0][1])
                pending = (it, Es)
            if pending is not None:
                ps_o = ot_mm(pending[0][0], pending[0][1], pending[1])
                ot_norm(pending[0][0], pending[0][1], ps_o)
                if not stop_after:
                    proj_phase(pending[0][1])

            if stop_after.startswith("I1") or stop_after == "I":
                _dump(O_all[:1] if stop_after.startswith("I1") else O_all)
                return


def host_prep(inputs):
    """Shared (per-core-identical) weight prep. Returns dict of np arrays."""
    f = np.float32
    q_w = np.asarray(inputs["q_w"], f)
    kv_w = np.asarray(inputs["kv_w"], f)
    proj_w = np.asarray(inputs["proj_w"], f)
    proj_b = np.asarray(inputs["proj_b"], f)
    dw_w = np.asarray(inputs["dw_w"], f)
    dw_b = np.asarray(inputs["dw_b"], f)
    bn_w = np.asarray(inputs["bn_w"], f)
    bn_b = np.asarray(inputs["bn_b"], f)
    bn_mean = np.asarray(inputs["bn_mean"], f)
    bn_var = np.asarray(inputs["bn_var"], f)
    pw_w = np.asarray(inputs["pw_w"], f)

    bn_s = (bn_w / np.sqrt(bn_var + BN_EPS)).astype(f)
    bn_t = ((dw_b - bn_mean) * bn_s + bn_b).astype(f)

    # wrap permutation: perm[p, 128j + 16b + q] = 1 iff p == 16j + q
    perm = np.zeros((LC, NLC, 8, 16), f)
    for j in range(NLC):
        for q in range(16):
            perm[16 * j + q, j, :, q] = 1.0
    perm = perm.reshape(LC, NLC * 128)

    p = np.arange(LC)
    c = np.arange(NLC)
    ytab_col = ((4 * c[None, :] + p[:, None] // 28 + 0.5 + 2.0 / A) * A).astype(f)
    ytab = np.tile(ytab_col, (1, G))                                   # [112, 21]
    xtab_col = ((p % 28 + 0.5 + 2.0 / A) * A).astype(f)[:, None]
    xtab = np.tile(xtab_col, (1, G * NLC))

    return {
        "qw_t": np.ascontiguousarray(q_w.T),
        "kwk_t": np.ascontiguousarray((kv_w[:DIM] * SCALE).T),
        "kwv_t": np.ascontiguousarray(kv_w[DIM:].T),
        "pw_t": np.ascontiguousarray(pw_w.T),
        "projw_t": np.ascontiguousarray(proj_w.T),
        "projb_rs": np.ascontiguousarray(proj_b.reshape(3, NGD).T),
        "dww": np.ascontiguousarray(dw_w.reshape(NGD, 25)),
        "bn_s": bn_s.reshape(NGD, 1),
        "bn_t": bn_t.reshape(NGD, 1),
        "ident": np.eye(128, dtype=f),
        "perm": perm,
        "ytab": ytab,
        "xtab": xtab,
    }


_NC_CACHE = {}


def _get_nc(gelu_exact=True):
    key = bool(gelu_exact)
    if key not in _NC_CACHE:
        _NC_CACHE[key] = build_nc(gelu_exact=key)
    return _NC_CACHE[key]


def make_in_maps(inputs):
    shared = host_prep(inputs)
    x = np.asarray(inputs["x"], np.float32)
    in_maps = []
    for i in range(B):
        m = dict(shared)
        m["x"] = np.ascontiguousarray(x[i].reshape(DIM, HW))
        in_maps.append(m)
    return in_maps


def run_spmd(inputs, trace=False):
    """Run on the 8 NeuronCores; returns (out (8,384,56,56), BassKernelResults)."""
    nc = _get_nc(True)
    in_maps = make_in_maps(inputs)
    res = bass_utils.run_bass_kernel_spmd(
        nc, in_maps, core_ids=list(range(B)), trace=trace,
    )
    out = np.stack([r["out"].reshape(DIM, H, W) for r in res.results], axis=0)
    return out, res


def kernel(**inputs) -> np.ndarray:
    out, _ = run_spmd(inputs, trace=False)
    return out
